# revision 15
# baseline (speedup 1.0000x reference)
"""Trainium2 Bass kernel for nn_AtomsGPT (GPT-2-style dense transformer).

B=4, T=1024, D=1024, H=16 heads, L=8 layers, V=50257, tied LM head.

Sharding (8 NeuronCores):
- Token-data-parallel trunk: core c owns batch c//2, pair-rank r=c%2.
  Rank r takes the even (r=0) / odd (r=1) 128-position tiles of the
  sequence, interleaved for causal-attention load balance.
- Per layer each core computes K/V only for its OWN 512 tokens and the
  pair exchanges K and V via two pipelined 1MB AllGathers (V first: it
  finishes during the LN window; K right after its matmuls).  The remote
  half is recovered bit-exactly as (bank0+bank1) - local in fp32, so the
  program stays rank-agnostic (k_all/v_all banks are core-relative:
  bank0=local, bank1=remote).  This removes the remote-K/V recompute
  matmuls entirely (~65K PE cycles/layer).
- Attention processes head PAIRS.  The two 64-wide score matmuls use
  disjoint PE row groups and run concurrently; both heads' scores land
  in one two-bank PSUM tile so a single Exp instruction (and a single
  mask multiply) covers the pair.  The per-pair Q matmuls for the second
  half of the heads are deferred into the attention stream to keep the
  PE busy (HAM stays un-throttled) while the ACT engine streams exp.
  Scores for iteration j+1 are emitted before AV of iteration j so the
  PE never waits on the exp latency.
- Softmax denominators ride in V's 65th column; both heads' denominators
  are broadcast with ONE matmul (stationary [2,128] 0/1 block matrix)
  and divided via a fast approx reciprocal.
- LayerNorm uses bn_stats/bn_aggr (one DVE pass for mean+var) and
  rstd = exp(-0.5*ln(var+eps)) so the whole kernel only ever loads the
  natural_log_exp activation table set plus gelu (2 overlapped switches
  per layer instead of 4 serial ones).
- The tied LM head is sharded over vocab (6288 cols/core); embedding
  weights stream chunk-by-chunk; the final world AllGather is split into
  token QUARTERS so head matmuls start on quarter 0 while the rest are
  in flight.

All matmuls run in bf16 with fp32 PSUM accumulation; the residual stream
and layernorm statistics stay fp32.  LN scales and the attention scale
are folded into weight matrices on the host (exact); all bias vectors in
this problem are structurally zero (asserted).
"""

import sys

for _p in ("/opt/trn_rl_repo", "/root/.axon_site"):
    if _p not in sys.path:
        sys.path.insert(0, _p)

import numpy as np
import ml_dtypes

import concourse.bass as bass
import concourse.tile as tile
from concourse import bacc, mybir
from concourse.bass_utils import run_bass_kernel_spmd
from concourse.hw_specs import get_activation_tables as _gat_orig


def _gat_patched(arch):
    """Blank the single-function ln/exp table sets so the compiler's
    first-match set selection lands on the combined
    natural_log_exp_and_others set. Otherwise every Ln->Exp pair in the
    layernorm rstd computation would reload the ACT tables (~2.7us each).
    Indices into act_info.json's act_func_sets are preserved."""
    t = _gat_orig(arch)
    if "natural_log_exp_and_others" in t:
        for n in ("exp_and_others", "natural_log"):
            if n in t:
                t[n] = set()
    return t


bacc.get_activation_tables = _gat_patched

F32 = mybir.dt.float32
BF16 = mybir.dt.bfloat16
AF = mybir.ActivationFunctionType
OP = mybir.AluOpType

B, T, D, H, L, V = 4, 1024, 1024, 16, 8, 50257
HD = D // H  # 64
EPS = 1e-5
N_CORES = 8
TOK = 512           # tokens per core
P = 128
VP = 6288           # per-core padded vocab slice (8*6288 = 50304 >= V)
NVC = (VP + 511) // 512  # vocab chunks (13; last is 144 wide)
PAIRS = [[0, 1], [2, 3], [4, 5], [6, 7]]
WORLD = [list(range(N_CORES))]


def positions_for_rank(r):
    """Global positions owned by pair-rank r, in local order (increasing)."""
    tiles = [2 * j + r for j in range(4)]
    return np.concatenate([np.arange(128 * t, 128 * (t + 1)) for t in tiles])


def _ln_tile(nc, sb, stat, t, x_ap, eps, xn_out):
    """LayerNorm stats + normalize for token tile t of x_ap [128,4,1024].
    Writes normalized bf16 (token-major) into xn_out [128,1024].
    Scale/bias are folded into downstream weights on the host."""
    stats = stat.tile([128, 2, 6], F32, tag="bnst")
    mv = stat.tile([128, 2], F32, tag="mv")
    xg = x_ap[:, t, :].rearrange("p (s f) -> p s f", s=2)
    for s in range(2):
        nc.vector.bn_stats(stats[:, s, :], xg[:, s, :])
    nc.vector.bn_aggr(mv[:], stats[:])
    lnv = stat.tile([128, 1], F32, tag="lnv")
    rstd = stat.tile([128, 1], F32, tag="rstd")
    nmr = stat.tile([128, 1], F32, tag="nmr")
    # rstd = (var+eps)^-0.5 via ln+exp: keeps ACT on the ln/exp table set
    nc.scalar.activation(lnv[:], mv[:, 1:2], AF.Ln, bias=eps[:])
    nc.scalar.activation(rstd[:], lnv[:], AF.Exp, scale=-0.5)
    nc.vector.tensor_mul(nmr[:], mv[:, 0:1], rstd[:])
    nc.vector.tensor_scalar_mul(nmr[:], nmr[:], -1.0)
    nc.vector.tensor_scalar(xn_out[:], x_ap[:, t, :], rstd[:], nmr[:],
                            OP.mult, OP.add)


def _ln_transpose(nc, psT, t, xn, xn_fm, ident, evac_eng):
    """Transpose token-major xn [128,1024] bf16 into feature-major
    xn_fm[:, :, t*128:(t+1)*128] via PE transposes + one evac copy.
    Shares the scores tag so trunk PSUM stays within 8 banks."""
    ptr = psT.tile([128, 8, 128], BF16, tag="sc")
    for kk in range(8):
        nc.tensor.transpose(ptr[:, kk, :], xn[:, kk * 128:(kk + 1) * 128], ident[:])
    if evac_eng == 0:
        nc.vector.tensor_copy(xn_fm[:, :, t * 128:(t + 1) * 128], ptr[:])
    else:
        nc.scalar.copy(xn_fm[:, :, t * 128:(t + 1) * 128], ptr[:])


def build(n_layers=L, dbg=False, no_cc=False):
    nc = bacc.Bacc("TRN2", target_bir_lowering=False, debug=False,
                   num_devices=N_CORES)

    x0_h = nc.dram_tensor("x0", [TOK, D], F32, kind="ExternalInput")
    # pre-tiled weights: [chunks, 128(p), 8(kk), 512(c)] per layer
    wqkv_h = nc.dram_tensor("wqkv", [n_layers, 6, 128, 8, 512], BF16, kind="ExternalInput")
    wp_h = nc.dram_tensor("wp", [n_layers, 2, 128, 8, 512], BF16, kind="ExternalInput")
    w1_h = nc.dram_tensor("w1", [n_layers, 8, 128, 8, 512], BF16, kind="ExternalInput")
    w2_h = nc.dram_tensor("w2", [n_layers, 2, 8, 128, 4, 512], BF16, kind="ExternalInput")
    embT_h = nc.dram_tensor("embT", [NVC, 128, 8, 512], BF16, kind="ExternalInput")
    msk_h = nc.dram_tensor("msk", [2, 128, 128], BF16, kind="ExternalInput")
    ident_h = nc.dram_tensor("identin", [128, 128], BF16, kind="ExternalInput")
    onesb_h = nc.dram_tensor("onesb", [1, 64], BF16, kind="ExternalInput")
    out_h = nc.dram_tensor("out", [N_CORES * TOK, VP], BF16, kind="ExternalOutput")

    dbg_outs = {}

    def dbg_dump(name, ap, shape, rearr=None):
        if not dbg:
            return
        tns = nc.dram_tensor(f"dbg_{name}", list(shape), ap.dtype, kind="ExternalOutput")
        dst = tns.ap() if rearr is None else tns.ap().rearrange(rearr)
        nc.sync.dma_start(dst, ap)
        dbg_outs[name] = shape

    # per-layer pair AllGather buffers for K and V (1MB in, 2MB out each)
    agk_in = [nc.dram_tensor(f"agkin{l}", [128, 4096], BF16, kind="Internal")
              for l in range(n_layers)]
    agk_out = [nc.dram_tensor(f"agkout{l}", [256, 4096], BF16, kind="Internal")
               for l in range(n_layers)]
    agv_in = [nc.dram_tensor(f"agvin{l}", [128, 4096], BF16, kind="Internal")
              for l in range(n_layers)]
    agv_out = [nc.dram_tensor(f"agvout{l}", [256, 4096], BF16, kind="Internal")
               for l in range(n_layers)]
    # final world AllGather split into token halves
    agf_in = [nc.dram_tensor(f"agfin{q}", [128, 2048], BF16, kind="Internal")
              for q in range(2)]
    agf_out = [nc.dram_tensor(f"agfout{q}", [N_CORES * 128, 2048], BF16,
                              kind="Internal", addr_space="Shared")
               for q in range(2)]

    with tile.TileContext(nc) as tc:
      with tc.tile_pool(name="const", bufs=1) as constp, \
           tc.tile_pool(name="xres", bufs=1) as xresp:
        with tc.tile_pool(name="stat", bufs=2) as stat, \
             tc.tile_pool(name="sb", bufs=2) as sb, \
             tc.tile_pool(name="act", bufs=1) as actp, \
             tc.tile_pool(name="wch", bufs=3) as wch, \
             tc.tile_pool(name="pp", bufs=4) as pp, \
             tc.tile_pool(name="rcn", bufs=2) as rcn, \
             tc.tile_pool(name="psM", bufs=2, space="PSUM") as psM, \
             tc.tile_pool(name="psS", bufs=2, space="PSUM") as psS, \
             tc.tile_pool(name="psV", bufs=2, space="PSUM") as psV:

            ident = constp.tile([128, 128], BF16)
            nc.sync.dma_start(ident[:], ident_h[:])
            # mask duplicated per head slot: msk2[:, b, h, :] = msk[b]
            msk2 = constp.tile([128, 2, 2, 128], BF16)
            for hdup in range(2):
                nc.sync.dma_start(msk2[:, :, hdup, :],
                                  msk_h.ap().rearrange("b p q -> p b q"))
            onesb = constp.tile([1, 64], BF16)
            nc.sync.dma_start(onesb[:], onesb_h[:])
            eps = constp.tile([128, 1], F32)
            nc.vector.memset(eps[:], EPS)

            # residual stream, token-major fp32 [part, tok-tile, D]
            x = xresp.tile([128, 4, D], F32)
            nc.sync.dma_start(x[:], x0_h.ap().rearrange("(t p) d -> p t d", p=128))

            for l in range(n_layers):
                # ---- LN1 (per tile) -> xn_fm; V matmuls for tiles 0/1 are
                # interleaved per tile as PE filler under the LN chain
                xn_fm = actp.tile([128, 8, TOK], BF16, tag="xn_fm")
                v_all = actp.tile([128, 2, 4, H, HD + 1], BF16, tag="v_all")
                nc.vector.memset(v_all[:, :, :, :, HD:HD + 1], 1.0)
                wvt = {}
                for ch in range(2):
                    wt = wch.tile([128, 8, 512], BF16, tag="w", name=f"wv{l}_{ch}")
                    nc.sync.dma_start(wt[:], wqkv_h[l, 4 + ch])
                    wvt[ch] = wt

                def v_tile(t):
                    for ch in range(2):
                        ps = psM.tile([128, TOK], F32, tag="mm")
                        for kk in range(8):
                            nc.tensor.matmul(
                                ps[:], xn_fm[:, kk, t * 128:(t + 1) * 128],
                                wvt[ch][:, kk, :], start=(kk == 0), stop=(kk == 7))
                        nc.vector.tensor_copy(
                            v_all[:, 0, t, ch * 8:(ch + 1) * 8, 0:HD],
                            ps[:].rearrange("p (h d) -> p h d", h=8))

                for t in range(4):
                    xn = sb.tile([128, 1024], BF16, tag="ln_xn")
                    _ln_tile(nc, sb, stat, t, x, eps, xn)
                    _ln_transpose(nc, psS, t, xn, xn_fm, ident, evac_eng=t % 2)
                    if t < 2:
                        v_tile(t)
                if l == 0:
                    dbg_dump("xn_fm0", xn_fm[:], [128, 8, TOK])

                # ---- remaining V tiles, then AG-V FIRST on the serial CC
                # stream (its payload is ready earliest; it overlaps the
                # K matmuls and AG-K follows right after)
                v_tile(2)
                v_tile(3)
                nc.sync.dma_start(
                    agv_in[l].ap().rearrange("p (t h d) -> p t h d", t=4, h=16),
                    v_all[:, 0, :, :, 0:HD])
                if no_cc:
                    nc.sync.dma_start(agv_out[l][0:128, :], agv_in[l][:])
                    nc.sync.dma_start(agv_out[l][128:256, :], agv_in[l][:])
                else:
                    nc.gpsimd.collective_compute(
                        "AllGather", OP.bypass, replica_groups=PAIRS,
                        ins=[agv_in[l][:]], outs=[agv_out[l][:]])

                # ---- K for LOCAL tokens -> AG-K (second on the CC stream;
                # its recon gates the bank-1 score matmuls)
                k_all = actp.tile([128, 2, 8, TOK], BF16, tag="k_all")
                for ch in range(2):
                    wt = wch.tile([128, 8, 512], BF16, tag="w", name=f"wk{l}_{ch}")
                    nc.sync.dma_start(wt[:], wqkv_h[l, 2 + ch])
                    for mi in range(4):
                        ps = psM.tile([128, TOK], F32, tag="mm")
                        for kk in range(8):
                            nc.tensor.matmul(
                                ps[:], wt[:, kk, mi * 128:(mi + 1) * 128],
                                xn_fm[:, kk, :], start=(kk == 0), stop=(kk == 7))
                        nc.vector.tensor_copy(k_all[:, 0, ch * 4 + mi, :], ps[:])
                nc.sync.dma_start(
                    agk_in[l].ap().rearrange("p (kk t) -> p kk t", kk=8),
                    k_all[:, 0])
                if no_cc:
                    nc.sync.dma_start(agk_out[l][0:128, :], agk_in[l][:])
                    nc.sync.dma_start(agk_out[l][128:256, :], agk_in[l][:])
                else:
                    nc.gpsimd.collective_compute(
                        "AllGather", OP.bypass, replica_groups=PAIRS,
                        ins=[agk_in[l][:]], outs=[agk_out[l][:]])

                # ---- recover remote K first (gates scores), then V, as
                # (bank0 + bank1) - local (fp32 intermediate => exact)
                for kk in range(8):
                    kb = rcn.tile([128, 2, TOK], BF16, tag="agb")
                    nc.sync.dma_start(
                        kb[:], agk_out[l].ap().rearrange(
                            "(b p) (kk t) -> p b kk t", p=128, kk=8)[:, :, kk])
                    ktmp = rcn.tile([128, TOK], F32, tag="tmp")
                    nc.vector.tensor_add(ktmp[:], kb[:, 0], kb[:, 1])
                    nc.vector.tensor_sub(k_all[:, 1, kk, :], ktmp[:],
                                         k_all[:, 0, kk, :])
                for t in range(4):
                    vb = rcn.tile([128, 2, H, HD], BF16, tag="agb")
                    nc.sync.dma_start(
                        vb[:], agv_out[l].ap().rearrange(
                            "(b p) (t h d) -> p b t h d", p=128, t=4, h=16)[:, :, t])
                    vtmp = rcn.tile([128, H, HD], F32, tag="tmp")
                    nc.vector.tensor_add(vtmp[:], vb[:, 0], vb[:, 1])
                    nc.vector.tensor_sub(v_all[:, 1, t, :, 0:HD], vtmp[:],
                                         v_all[:, 0, t, :, 0:HD])

                # ---- Q for the first 4 head pairs (the rest are deferred
                # into the attention stream as PE filler)
                q_fm = actp.tile([128, 8, TOK], BF16, tag="q_fm")
                wqt = {}
                for ch in range(2):
                    wt = wch.tile([128, 8, 512], BF16, tag="wq", bufs=2,
                                  name=f"wq{l}_{ch}")
                    nc.sync.dma_start(wt[:], wqkv_h[l, ch])
                    wqt[ch] = wt

                def q_pair(hp):
                    ch, mi = hp // 4, hp % 4
                    ps = psM.tile([128, TOK], F32, tag="mm")
                    for kk in range(8):
                        nc.tensor.matmul(ps[:], wqt[ch][:, kk, mi * 128:(mi + 1) * 128],
                                         xn_fm[:, kk, :], start=(kk == 0), stop=(kk == 7))
                    nc.vector.tensor_copy(q_fm[:, hp, :], ps[:])

                for hp in range(4):
                    q_pair(hp)

                # ---- attention: flat software-pipelined loop over
                # (pair, bank, kt-tile); scores for j+1 are emitted before
                # AV of j so the PE never waits on exp
                o_fm = actp.tile([128, 8, TOK], BF16, tag="o_fm")
                av = {}

                def emit_scores(hp, b, i):
                    n = TOK - 128 * i
                    sc = psS.tile([128, 2, TOK], F32, tag="sc")
                    nc.tensor.matmul(
                        sc[:, 0, 0:n], k_all[0:64, b, hp, i * 128:(i + 1) * 128],
                        q_fm[0:64, hp, 128 * i:TOK], start=True, stop=True)
                    nc.tensor.matmul(
                        sc[:, 1, 0:n], k_all[64:128, b, hp, i * 128:(i + 1) * 128],
                        q_fm[64:128, hp, 128 * i:TOK], start=True, stop=True)
                    pt = pp.tile([128, 2, TOK], BF16, tag="p")
                    nc.scalar.activation(pt[:, :, 0:n], sc[:, :, 0:n], AF.Exp)
                    nc.vector.tensor_mul(pt[:, :, 0:128], pt[:, :, 0:128],
                                         msk2[:, b, :, :])
                    return pt

                def emit_av(hp, b, i, pt):
                    n = TOK - 128 * i
                    if b == 0 and i == 0:
                        av[hp] = (psV.tile([128, TOK], F32, tag="av", name=f"ave{l}_{hp}"),
                                  psV.tile([128, TOK], F32, tag="av", name=f"avo{l}_{hp}"))
                    ave, avo = av[hp]
                    st = (b == 0 and i == 0)
                    sp = (b == 1 and i == 3)
                    nc.tensor.matmul(ave[0:HD + 1, 128 * i:TOK],
                                     v_all[:, b, i, 2 * hp, :], pt[:, 0, 0:n],
                                     start=st, stop=sp)
                    nc.tensor.matmul(avo[0:HD + 1, 128 * i:TOK],
                                     v_all[:, b, i, 2 * hp + 1, :], pt[:, 1, 0:n],
                                     start=st, stop=sp)

                def finish_pair(hp):
                    ave, avo = av.pop(hp)
                    den2 = sb.tile([1, 2, TOK], BF16, tag="den")
                    nc.vector.tensor_copy(den2[0:1, 0, :], ave[HD:HD + 1, :])
                    nc.vector.tensor_copy(den2[0:1, 1, :], avo[HD:HD + 1, :])
                    bp = psM.tile([128, TOK], F32, tag="mm")
                    nc.tensor.matmul(bp[0:64, :], onesb[:], den2[0:1, 0, :],
                                     start=True, stop=True)
                    nc.tensor.matmul(bp[64:128, :], onesb[:], den2[0:1, 1, :],
                                     start=True, stop=True)
                    rb = sb.tile([128, TOK], F32, tag="rb")
                    nc.vector.reciprocal_approx_fast(rb[:], bp[:])
                    nc.vector.tensor_tensor(o_fm[0:64, hp, :], ave[0:HD, :],
                                            rb[0:64, :], OP.mult)
                    nc.vector.tensor_tensor(o_fm[64:128, hp, :], avo[0:HD, :],
                                            rb[64:128, :], OP.mult)

                iters = [(hp, b, i) for hp in range(H // 2)
                         for b in range(2) for i in range(4)]
                prev = None
                for j, (hp, b, i) in enumerate(iters):
                    if b == 0 and i == 0 and 3 <= hp <= 6:
                        q_pair(hp + 1)  # deferred Q as PE filler, 1 pair ahead
                    pt = emit_scores(hp, b, i)
                    if prev is not None:
                        emit_av(*prev)
                        if prev[1] == 1 and prev[2] == 3:
                            finish_pair(prev[0])
                    prev = (hp, b, i, pt)
                emit_av(*prev)
                finish_pair(prev[0])

                if l == 0:
                    dbg_dump("k_all0", k_all[:], [128, 2, 8, TOK])
                    dbg_dump("v_all0", v_all[:], [128, 2, 4, H, HD + 1])
                    dbg_dump("o_fm0", o_fm[:], [128, 8, TOK])

                # ---- projection (token-major, t-outer) + residual + LN2(t)
                xn2_fm = actp.tile([128, 8, TOK], BF16, tag="xn2_fm")
                wpt = {}
                for ch in range(2):
                    wt = wch.tile([128, 8, 512], BF16, tag="w", name=f"wpj{l}_{ch}")
                    nc.sync.dma_start(wt[:], wp_h[l, ch])
                    wpt[ch] = wt
                for t in range(4):
                    for ch in range(2):
                        ps = psM.tile([128, 512], F32, tag="mm")
                        for kk in range(8):
                            nc.tensor.matmul(
                                ps[:], o_fm[:, kk, t * 128:(t + 1) * 128],
                                wpt[ch][:, kk, :], start=(kk == 0), stop=(kk == 7))
                        nc.vector.tensor_add(x[:, t, ch * 512:(ch + 1) * 512],
                                             x[:, t, ch * 512:(ch + 1) * 512], ps[:])
                    xn2 = sb.tile([128, 1024], BF16, tag="ln_xn")
                    _ln_tile(nc, sb, stat, t, x, eps, xn2)
                    _ln_transpose(nc, psS, t, xn2, xn2_fm, ident, evac_eng=(t + 1) % 2)
                if l == 0:
                    dbg_dump("xattn0", x[:], [128, 4, D])

                # ---- FFN: ff1 full-token, ff2 in token halves
                h_sb = actp.tile([128, 32, TOK], BF16, tag="h_sb")
                for mc in range(8):
                    wt = wch.tile([128, 8, 512], BF16, tag="w", name=f"w1_{l}_{mc}")
                    nc.sync.dma_start(wt[:], w1_h[l, mc])
                    for mi in range(4):
                        ps = psM.tile([128, TOK], F32, tag="mm")
                        for kk in range(8):
                            nc.tensor.matmul(
                                ps[:], wt[:, kk, mi * 128:(mi + 1) * 128],
                                xn2_fm[:, kk, :], start=(kk == 0), stop=(kk == 7))
                        nc.scalar.activation(h_sb[:, mc * 4 + mi, :], ps[:], AF.Gelu)
                for half in range(2):
                    for nch in range(2):
                        acc = [psV.tile([128, 512], F32, tag="av",
                                        name=f"acc{l}_{half}_{nch}_{a}") for a in range(2)]
                        for kkc in range(8):
                            w2t = wch.tile([128, 4, 512], BF16, tag="w",
                                           name=f"w2_{l}_{half}_{nch}_{kkc}")
                            nc.sync.dma_start(w2t[:], w2_h[l, nch, kkc])
                            for kki in range(4):
                                for mi in range(2):
                                    nc.tensor.matmul(
                                        acc[mi][:],
                                        h_sb[:, kkc * 4 + kki,
                                             half * 256 + mi * 128:half * 256 + (mi + 1) * 128],
                                        w2t[:, kki, :],
                                        start=(kkc == 0 and kki == 0),
                                        stop=(kkc == 7 and kki == 3))
                        for mi in range(2):
                            t = half * 2 + mi
                            nc.vector.tensor_add(x[:, t, nch * 512:(nch + 1) * 512],
                                                 x[:, t, nch * 512:(nch + 1) * 512],
                                                 acc[mi][:])
                if l == 0:
                    dbg_dump("xlayer0", x[:], [128, 4, D])

        # ---- final LN + LM head phase (separate pools; trunk SBUF released)
        with tc.tile_pool(name="stat2", bufs=2) as stat2, \
             tc.tile_pool(name="sb2", bufs=2) as sb2, \
             tc.tile_pool(name="hd", bufs=1) as hd, \
             tc.tile_pool(name="emb", bufs=3) as epool, \
             tc.tile_pool(name="hout", bufs=4) as hout, \
             tc.tile_pool(name="psT2", bufs=2, space="PSUM") as psT2, \
             tc.tile_pool(name="psH", bufs=4, space="PSUM") as psH:
            eps2 = sb2.tile([128, 1], F32, tag="eps", bufs=1)
            nc.vector.memset(eps2[:], EPS)
            xnf_fm = hd.tile([128, 8, TOK], BF16)
            # final LN + world AllGather per token half so head matmuls
            # start on half 0 while half 1 is still in flight
            for t in range(4):
                xnf = sb2.tile([128, 1024], BF16, tag="ln_xn")
                _ln_tile(nc, sb2, stat2, t, x, eps2, xnf)
                _ln_transpose(nc, psT2, t, xnf, xnf_fm, ident, evac_eng=t % 2)
                if t % 2 == 1:
                    hh = t // 2
                    nc.sync.dma_start(
                        agf_in[hh].ap().rearrange("p (kk t1) -> p kk t1", kk=8),
                        xnf_fm[:, :, hh * 256:(hh + 1) * 256])
                    if no_cc:
                        for r_ in range(N_CORES):
                            nc.sync.dma_start(
                                agf_out[hh][r_ * 128:(r_ + 1) * 128, :], agf_in[hh][:])
                    else:
                        nc.gpsimd.collective_compute(
                            "AllGather", OP.bypass, replica_groups=WORLD,
                            ins=[agf_in[hh][:]], outs=[agf_out[hh][:]])
            if dbg:
                dbg_dump("xnf_fm", xnf_fm[:], [128, 8, TOK])
            xn_all = hd.tile([128, 64, TOK], BF16)
            for hh in range(2):
                for r_ in range(8):
                    nc.sync.dma_start(
                        xn_all[:, r_ * 8:(r_ + 1) * 8, hh * 256:(hh + 1) * 256],
                        agf_out[hh][r_ * 128:(r_ + 1) * 128, :].rearrange(
                            "p (kk t1) -> p kk t1", kk=8))
            nchunks = [(i * 512, min(512, VP - i * 512)) for i in range(NVC)]
            for ni, (n0, nsz) in enumerate(nchunks):
                et = epool.tile([128, 8, 512], BF16, tag="emb")
                nc.sync.dma_start(et[:], embT_h[ni])
                # token quarter 0 (tile 0 of every rank) first: available as
                # soon as the first quarter-AG lands
                for t in range(4):
                    for r_ in range(8):
                        mi = r_ * 4 + t
                        ps = psH.tile([128, nsz], F32, tag="h")
                        for kk in range(8):
                            nc.tensor.matmul(
                                ps[:], xn_all[:, r_ * 8 + kk, t * 128:(t + 1) * 128],
                                et[:, kk, 0:nsz],
                                start=(kk == 0), stop=(kk == 7))
                        osb = hout.tile([128, nsz], BF16, tag="o")
                        if (t * 8 + r_) % 2 == 0:
                            nc.vector.tensor_copy(osb[:], ps[:])
                        else:
                            nc.scalar.copy(osb[:], ps[:])
                        nc.sync.dma_start(out_h[mi * 128:(mi + 1) * 128, n0:n0 + nsz], osb[:])

    nc.compile()
    return nc, dbg_outs


def _fm_tile_w(w):
    """[1024, nch*512] -> [nch, 128, 8, 512]; tile[j,p,kk,c] = w[kk*128+p, j*512+c]."""
    din, dout = w.shape
    nch = dout // 512
    r = w.reshape(8, 128, nch, 512)
    return np.ascontiguousarray(r.transpose(2, 1, 0, 3))


def prepare_inputs(idx, tok_emb, pos_emb, qkv_w, qkv_b, proj_w, proj_b,
                   ff1_w, ff1_b, ff2_w, ff2_b, ln1_s, ln1_b, ln2_s, ln2_b,
                   lnf_s, lnf_b, n_layers=L):
    """Host-side sharding/folding. Returns per-core in_maps."""
    bf = ml_dtypes.bfloat16
    for name, v in (("qkv_b", qkv_b), ("proj_b", proj_b), ("ff1_b", ff1_b),
                    ("ff2_b", ff2_b), ("ln1_b", ln1_b), ("ln2_b", ln2_b),
                    ("lnf_b", lnf_b)):
        assert np.allclose(np.asarray(v), 0.0), f"nonzero {name} not supported"

    idx = np.asarray(idx)
    tok_emb = np.asarray(tok_emb, np.float32)
    pos_emb = np.asarray(pos_emb, np.float32)
    scale = 1.0 / np.sqrt(HD)

    # fold LN scales + attention scale into weights (exact)
    wqkv = (np.asarray(qkv_w[:n_layers], np.float32)
            * np.asarray(ln1_s[:n_layers], np.float32)[:, :, None]).copy()
    wqkv[:, :, :D] *= scale
    w1 = (np.asarray(ff1_w[:n_layers], np.float32)
          * np.asarray(ln2_s[:n_layers], np.float32)[:, :, None])
    wp = np.asarray(proj_w[:n_layers], np.float32)
    w2 = np.asarray(ff2_w[:n_layers], np.float32)
    embT_full = (tok_emb * np.asarray(lnf_s, np.float32)[None, :]).T  # [D, V]
    embT_pad = np.zeros((D, N_CORES * VP), np.float32)
    embT_pad[:, :V] = embT_full

    # pre-tiled weight arrays (contiguous 1MB DMA bursts on device)
    wqkv_t = np.stack([_fm_tile_w(wqkv[l]) for l in range(n_layers)]).astype(bf)
    wp_t = np.stack([_fm_tile_w(wp[l]) for l in range(n_layers)]).astype(bf)
    w1_t = np.stack([_fm_tile_w(w1[l]) for l in range(n_layers)]).astype(bf)
    # w2: [4096, 1024] -> [2(nch), 8(kkc), 128(p), 4(kki), 512(c)]
    w2_t = np.stack([
        np.ascontiguousarray(
            w2[l].reshape(8, 4, 128, 2, 512).transpose(3, 0, 2, 1, 4))
        for l in range(n_layers)]).astype(bf)

    ident = np.eye(128, dtype=bf)
    onesb = np.ones((1, 64), bf)

    tri = np.tril(np.ones((128, 128), np.float32)).T  # [kt, q] valid kt<=q
    # core-relative banks: slot0 = local diagonal (triangular for both
    # ranks); slot1 = remote diagonal (all-masked for r=0, visible for r=1)
    msk_r = [np.zeros((2, 128, 128), np.float32) for _ in range(2)]
    msk_r[0][0] = tri
    msk_r[0][1] = 0.0
    msk_r[1][0] = tri
    msk_r[1][1] = 1.0

    in_maps = []
    for c in range(N_CORES):
        b, r = c // 2, c % 2
        pos = positions_for_rank(r)
        x0 = tok_emb[idx[b, pos]] + pos_emb[pos]
        # per-core vocab slice, padded to 13*512 cols for uniform DMA
        esl = np.zeros((D, NVC * 512), np.float32)
        esl[:, :VP] = embT_pad[:, c * VP:(c + 1) * VP]
        embT_tiles = np.ascontiguousarray(
            esl.reshape(8, 128, NVC, 512).transpose(2, 1, 0, 3)).astype(bf)
        in_maps.append({
            "x0": np.ascontiguousarray(x0, np.float32),
            "wqkv": wqkv_t, "wp": wp_t, "w1": w1_t, "w2": w2_t,
            "embT": embT_tiles,
            "msk": msk_r[r].astype(bf),
            "identin": ident,
            "onesb": onesb,
        })
    return in_maps


def assemble_output(results):
    """Per-core [4096, VP] bf16 -> full logits [B, T, V] f32."""
    logits = np.empty((B, T, V), np.float32)
    pos_r = [positions_for_rank(0), positions_for_rank(1)]
    for c in range(N_CORES):
        out_c = np.asarray(results[c]["out"], np.float32)  # [4096, VP]
        v0 = c * VP
        ncols = min(VP, V - v0)
        if ncols <= 0:
            continue
        for r in range(N_CORES):
            bb, rr = r // 2, r % 2
            logits[bb, pos_r[rr], v0:v0 + ncols] = \
                out_c[r * TOK:(r + 1) * TOK, :ncols]
    return logits


_NC_CACHE = {}


def _get_nc(n_layers=L, dbg=False):
    key = (n_layers, dbg)
    if key not in _NC_CACHE:
        _NC_CACHE[key] = build(n_layers=n_layers, dbg=dbg)
    return _NC_CACHE[key]


def kernel(**inputs):
    in_maps = prepare_inputs(**inputs)
    nc, _ = _get_nc()
    res = run_bass_kernel_spmd(nc, in_maps, core_ids=list(range(N_CORES)))
    return assemble_output(res.results)


# revision 16
# speedup vs baseline: 1.0545x; 1.0545x over previous
"""Trainium2 Bass kernel for nn_AtomsGPT (GPT-2-style dense transformer).

B=4, T=1024, D=1024, H=16 heads, L=8 layers, V=50257, tied LM head.

Sharding (8 NeuronCores):
- Token-data-parallel trunk: core c owns batch c//2, pair-rank r=c%2.
  Rank r takes the even (r=0) / odd (r=1) 128-position tiles of the
  sequence, interleaved for causal-attention load balance.
- Per layer each core computes K/V only for its OWN 512 tokens and the
  pair exchanges K and V via two pipelined 1MB AllGathers (V first: it
  finishes during the LN window; K right after its matmuls).  The remote
  half is recovered bit-exactly as (bank0+bank1) - local in fp32, so the
  program stays rank-agnostic (k_all/v_all banks are core-relative:
  bank0=local, bank1=remote).  This removes the remote-K/V recompute
  matmuls entirely (~65K PE cycles/layer).
- Attention processes head PAIRS.  The two 64-wide score matmuls use
  disjoint PE row groups and run concurrently; both heads' scores land
  in one two-bank PSUM tile so a single Exp instruction (and a single
  mask multiply) covers the pair.  The per-pair Q matmuls for the second
  half of the heads are deferred into the attention stream to keep the
  PE busy (HAM stays un-throttled) while the ACT engine streams exp.
  Scores for iteration j+1 are emitted before AV of iteration j so the
  PE never waits on the exp latency.
- Softmax denominators ride in V's 65th column; both heads' denominators
  are broadcast with ONE matmul (stationary [2,128] 0/1 block matrix)
  and divided via a fast approx reciprocal.
- LayerNorm uses bn_stats/bn_aggr (one DVE pass for mean+var) and
  rstd = exp(-0.5*ln(var+eps)) so the whole kernel only ever loads the
  natural_log_exp activation table set plus gelu (2 overlapped switches
  per layer instead of 4 serial ones).
- The tied LM head is sharded over vocab (6288 cols/core); embedding
  weights stream chunk-by-chunk; the final world AllGather is split into
  token QUARTERS so head matmuls start on quarter 0 while the rest are
  in flight.

All matmuls run in bf16 with fp32 PSUM accumulation; the residual stream
and layernorm statistics stay fp32.  LN scales and the attention scale
are folded into weight matrices on the host (exact); all bias vectors in
this problem are structurally zero (asserted).
"""

import sys

for _p in ("/opt/trn_rl_repo", "/root/.axon_site"):
    if _p not in sys.path:
        sys.path.insert(0, _p)

import numpy as np
import ml_dtypes

import concourse.bass as bass
import concourse.tile as tile
from concourse import bacc, mybir
from concourse.bass_utils import run_bass_kernel_spmd
from concourse.hw_specs import get_activation_tables as _gat_orig


def _gat_patched(arch):
    """Blank the single-function ln/exp table sets so the compiler's
    first-match set selection lands on the combined
    natural_log_exp_and_others set. Otherwise every Ln->Exp pair in the
    layernorm rstd computation would reload the ACT tables (~2.7us each).
    Indices into act_info.json's act_func_sets are preserved."""
    t = _gat_orig(arch)
    if "natural_log_exp_and_others" in t:
        for n in ("exp_and_others", "natural_log"):
            if n in t:
                t[n] = set()
    return t


bacc.get_activation_tables = _gat_patched

F32 = mybir.dt.float32
BF16 = mybir.dt.bfloat16
AF = mybir.ActivationFunctionType
OP = mybir.AluOpType

B, T, D, H, L, V = 4, 1024, 1024, 16, 8, 50257
HD = D // H  # 64
EPS = 1e-5
N_CORES = 8
TOK = 512           # tokens per core
P = 128
VP = 6288           # per-core padded vocab slice (8*6288 = 50304 >= V)
NVC = (VP + 511) // 512  # vocab chunks (13; last is 144 wide)
PAIRS = [[0, 1], [2, 3], [4, 5], [6, 7]]
WORLD = [list(range(N_CORES))]


def positions_for_rank(r):
    """Global positions owned by pair-rank r, in local order (increasing)."""
    tiles = [2 * j + r for j in range(4)]
    return np.concatenate([np.arange(128 * t, 128 * (t + 1)) for t in tiles])


def _ln_tile(nc, sb, stat, t, x_ap, eps, xn_out):
    """LayerNorm stats + normalize for token tile t of x_ap [128,4,1024].
    Writes normalized bf16 (token-major) into xn_out [128,1024].
    Scale/bias are folded into downstream weights on the host."""
    stats = stat.tile([128, 2, 6], F32, tag="bnst")
    mv = stat.tile([128, 2], F32, tag="mv")
    xg = x_ap[:, t, :].rearrange("p (s f) -> p s f", s=2)
    for s in range(2):
        nc.vector.bn_stats(stats[:, s, :], xg[:, s, :])
    nc.vector.bn_aggr(mv[:], stats[:])
    lnv = stat.tile([128, 1], F32, tag="lnv")
    rstd = stat.tile([128, 1], F32, tag="rstd")
    nmr = stat.tile([128, 1], F32, tag="nmr")
    # rstd = (var+eps)^-0.5 via ln+exp: keeps ACT on the ln/exp table set
    nc.scalar.activation(lnv[:], mv[:, 1:2], AF.Ln, bias=eps[:])
    nc.scalar.activation(rstd[:], lnv[:], AF.Exp, scale=-0.5)
    nc.vector.tensor_mul(nmr[:], mv[:, 0:1], rstd[:])
    nc.vector.tensor_scalar_mul(nmr[:], nmr[:], -1.0)
    nc.vector.tensor_scalar(xn_out[:], x_ap[:, t, :], rstd[:], nmr[:],
                            OP.mult, OP.add)


def _ln_transpose(nc, psT, t, xn, xn_fm, ident, evac_eng):
    """Transpose token-major xn [128,1024] bf16 into feature-major
    xn_fm[:, :, t*128:(t+1)*128] via PE transposes + one evac copy.
    Shares the scores tag so trunk PSUM stays within 8 banks."""
    ptr = psT.tile([128, 8, 128], BF16, tag="sc")
    for kk in range(8):
        nc.tensor.transpose(ptr[:, kk, :], xn[:, kk * 128:(kk + 1) * 128], ident[:])
    if evac_eng == 0:
        nc.vector.tensor_copy(xn_fm[:, :, t * 128:(t + 1) * 128], ptr[:])
    else:
        nc.scalar.copy(xn_fm[:, :, t * 128:(t + 1) * 128], ptr[:])


def build(n_layers=L, dbg=False, no_cc=False):
    nc = bacc.Bacc("TRN2", target_bir_lowering=False, debug=False,
                   num_devices=N_CORES)

    x0_h = nc.dram_tensor("x0", [TOK, D], F32, kind="ExternalInput")
    # pre-tiled weights: [chunks, 128(p), 8(kk), 512(c)] per layer
    wqkv_h = nc.dram_tensor("wqkv", [n_layers, 6, 128, 8, 512], BF16, kind="ExternalInput")
    wp_h = nc.dram_tensor("wp", [n_layers, 2, 128, 8, 512], BF16, kind="ExternalInput")
    w1_h = nc.dram_tensor("w1", [n_layers, 8, 128, 8, 512], BF16, kind="ExternalInput")
    w2_h = nc.dram_tensor("w2", [n_layers, 2, 8, 128, 4, 512], BF16, kind="ExternalInput")
    embT_h = nc.dram_tensor("embT", [NVC, 128, 8, 512], BF16, kind="ExternalInput")
    msk_h = nc.dram_tensor("msk", [2, 128, 128], BF16, kind="ExternalInput")
    ident_h = nc.dram_tensor("identin", [128, 128], BF16, kind="ExternalInput")
    onesb_h = nc.dram_tensor("onesb", [1, 64], BF16, kind="ExternalInput")
    out_h = nc.dram_tensor("out", [N_CORES * TOK, VP], BF16, kind="ExternalOutput")

    dbg_outs = {}

    def dbg_dump(name, ap, shape, rearr=None):
        if not dbg:
            return
        tns = nc.dram_tensor(f"dbg_{name}", list(shape), ap.dtype, kind="ExternalOutput")
        dst = tns.ap() if rearr is None else tns.ap().rearrange(rearr)
        nc.sync.dma_start(dst, ap)
        dbg_outs[name] = shape

    # per-layer pair AllGather buffers for K and V (1MB in, 2MB out each)
    agk_in = [nc.dram_tensor(f"agkin{l}", [128, 4096], BF16, kind="Internal")
              for l in range(n_layers)]
    agk_out = [nc.dram_tensor(f"agkout{l}", [256, 4096], BF16, kind="Internal")
               for l in range(n_layers)]
    agv_in = [nc.dram_tensor(f"agvin{l}", [128, 4096], BF16, kind="Internal")
              for l in range(n_layers)]
    agv_out = [nc.dram_tensor(f"agvout{l}", [256, 4096], BF16, kind="Internal")
               for l in range(n_layers)]
    # final world AllGather split into token halves
    agf_in = [nc.dram_tensor(f"agfin{q}", [128, 2048], BF16, kind="Internal")
              for q in range(2)]
    agf_out = [nc.dram_tensor(f"agfout{q}", [N_CORES * 128, 2048], BF16,
                              kind="Internal", addr_space="Shared")
               for q in range(2)]

    with tile.TileContext(nc) as tc:
      with tc.tile_pool(name="const", bufs=1) as constp, \
           tc.tile_pool(name="xres", bufs=1) as xresp:
        with tc.tile_pool(name="stat", bufs=2) as stat, \
             tc.tile_pool(name="sb", bufs=2) as sb, \
             tc.tile_pool(name="act", bufs=1) as actp, \
             tc.tile_pool(name="wch", bufs=3) as wch, \
             tc.tile_pool(name="pp", bufs=4) as pp, \
             tc.tile_pool(name="rcn", bufs=2) as rcn, \
             tc.tile_pool(name="psM", bufs=2, space="PSUM") as psM, \
             tc.tile_pool(name="psS", bufs=2, space="PSUM") as psS, \
             tc.tile_pool(name="psV", bufs=2, space="PSUM") as psV:

            ident = constp.tile([128, 128], BF16)
            nc.sync.dma_start(ident[:], ident_h[:])
            # mask duplicated per head slot: msk2[:, b, h, :] = msk[b]
            msk2 = constp.tile([128, 2, 2, 128], BF16)
            for hdup in range(2):
                nc.sync.dma_start(msk2[:, :, hdup, :],
                                  msk_h.ap().rearrange("b p q -> p b q"))
            onesb = constp.tile([1, 64], BF16)
            nc.sync.dma_start(onesb[:], onesb_h[:])
            eps = constp.tile([128, 1], F32)
            nc.vector.memset(eps[:], EPS)

            # residual stream, token-major fp32 [part, tok-tile, D]
            x = xresp.tile([128, 4, D], F32)
            nc.sync.dma_start(x[:], x0_h.ap().rearrange("(t p) d -> p t d", p=128))

            for l in range(n_layers):
                # ---- LN1 (per tile) -> xn_fm (feature-major bf16)
                xn_fm = actp.tile([128, 8, TOK], BF16, tag="xn_fm")
                for t in range(4):
                    xn = sb.tile([128, 1024], BF16, tag="ln_xn")
                    _ln_tile(nc, sb, stat, t, x, eps, xn)
                    _ln_transpose(nc, psS, t, xn, xn_fm, ident, evac_eng=t % 2)
                if l == 0:
                    dbg_dump("xn_fm0", xn_fm[:], [128, 8, TOK])

                # ---- V for LOCAL tokens (bank 0) -> AG-V first on the
                # serial CC stream (payload ready earliest; overlaps K)
                v_all = actp.tile([128, 2, 4, H, HD + 1], BF16, tag="v_all")
                nc.vector.memset(v_all[:, :, :, :, HD:HD + 1], 1.0)
                for ch in range(2):
                    wt = wch.tile([128, 8, 512], BF16, tag="w", name=f"wv{l}_{ch}")
                    nc.sync.dma_start(wt[:], wqkv_h[l, 4 + ch])
                    for t in range(4):
                        ps = psM.tile([128, TOK], F32, tag="mm")
                        for kk in range(8):
                            nc.tensor.matmul(
                                ps[:], xn_fm[:, kk, t * 128:(t + 1) * 128],
                                wt[:, kk, :], start=(kk == 0), stop=(kk == 7))
                        nc.vector.tensor_copy(
                            v_all[:, 0, t, ch * 8:(ch + 1) * 8, 0:HD],
                            ps[:].rearrange("p (h d) -> p h d", h=8))
                nc.sync.dma_start(
                    agv_in[l].ap().rearrange("p (t h d) -> p t h d", t=4, h=16),
                    v_all[:, 0, :, :, 0:HD])
                if no_cc:
                    nc.sync.dma_start(agv_out[l][0:128, :], agv_in[l][:])
                    nc.sync.dma_start(agv_out[l][128:256, :], agv_in[l][:])
                else:
                    nc.gpsimd.collective_compute(
                        "AllGather", OP.bypass, replica_groups=PAIRS,
                        ins=[agv_in[l][:]], outs=[agv_out[l][:]])

                # ---- K for LOCAL tokens -> AG-K (second on the CC stream;
                # its recon gates the bank-1 score matmuls)
                k_all = actp.tile([128, 2, 8, TOK], BF16, tag="k_all")
                for ch in range(2):
                    wt = wch.tile([128, 8, 512], BF16, tag="w", name=f"wk{l}_{ch}")
                    nc.sync.dma_start(wt[:], wqkv_h[l, 2 + ch])
                    for mi in range(4):
                        ps = psM.tile([128, TOK], F32, tag="mm")
                        for kk in range(8):
                            nc.tensor.matmul(
                                ps[:], wt[:, kk, mi * 128:(mi + 1) * 128],
                                xn_fm[:, kk, :], start=(kk == 0), stop=(kk == 7))
                        nc.vector.tensor_copy(k_all[:, 0, ch * 4 + mi, :], ps[:])
                nc.sync.dma_start(
                    agk_in[l].ap().rearrange("p (kk t) -> p kk t", kk=8),
                    k_all[:, 0])
                if no_cc:
                    nc.sync.dma_start(agk_out[l][0:128, :], agk_in[l][:])
                    nc.sync.dma_start(agk_out[l][128:256, :], agk_in[l][:])
                else:
                    nc.gpsimd.collective_compute(
                        "AllGather", OP.bypass, replica_groups=PAIRS,
                        ins=[agk_in[l][:]], outs=[agk_out[l][:]])

                # ---- recover remote V then K bit-exactly as
                # (bank0 + bank1) - local (fp32 intermediate => exact)
                for t in range(4):
                    vb = rcn.tile([128, 2, H, HD], BF16, tag="agb")
                    nc.sync.dma_start(
                        vb[:], agv_out[l].ap().rearrange(
                            "(b p) (t h d) -> p b t h d", p=128, t=4, h=16)[:, :, t])
                    vtmp = rcn.tile([128, H, HD], F32, tag="tmp")
                    nc.vector.tensor_add(vtmp[:], vb[:, 0], vb[:, 1])
                    nc.vector.tensor_sub(v_all[:, 1, t, :, 0:HD], vtmp[:],
                                         v_all[:, 0, t, :, 0:HD])
                for kk in range(8):
                    kb = rcn.tile([128, 2, TOK], BF16, tag="agb")
                    nc.sync.dma_start(
                        kb[:], agk_out[l].ap().rearrange(
                            "(b p) (kk t) -> p b kk t", p=128, kk=8)[:, :, kk])
                    ktmp = rcn.tile([128, TOK], F32, tag="tmp")
                    nc.vector.tensor_add(ktmp[:], kb[:, 0], kb[:, 1])
                    nc.vector.tensor_sub(k_all[:, 1, kk, :], ktmp[:],
                                         k_all[:, 0, kk, :])

                # ---- Q for the first 4 head pairs (the rest are deferred
                # into the attention stream as PE filler)
                q_fm = actp.tile([128, 8, TOK], BF16, tag="q_fm")
                wqt = {}
                for ch in range(2):
                    wt = wch.tile([128, 8, 512], BF16, tag="wq", bufs=2,
                                  name=f"wq{l}_{ch}")
                    nc.sync.dma_start(wt[:], wqkv_h[l, ch])
                    wqt[ch] = wt

                def q_pair(hp):
                    ch, mi = hp // 4, hp % 4
                    ps = psM.tile([128, TOK], F32, tag="mm")
                    for kk in range(8):
                        nc.tensor.matmul(ps[:], wqt[ch][:, kk, mi * 128:(mi + 1) * 128],
                                         xn_fm[:, kk, :], start=(kk == 0), stop=(kk == 7))
                    nc.vector.tensor_copy(q_fm[:, hp, :], ps[:])

                for hp in range(4):
                    q_pair(hp)

                # ---- attention: flat software-pipelined loop over
                # (pair, bank, kt-tile); scores for j+1 are emitted before
                # AV of j so the PE never waits on exp
                o_fm = actp.tile([128, 8, TOK], BF16, tag="o_fm")
                av = {}

                def emit_scores(hp, b, i):
                    n = TOK - 128 * i
                    sc = psS.tile([128, 2, TOK], F32, tag="sc")
                    nc.tensor.matmul(
                        sc[:, 0, 0:n], k_all[0:64, b, hp, i * 128:(i + 1) * 128],
                        q_fm[0:64, hp, 128 * i:TOK], start=True, stop=True)
                    nc.tensor.matmul(
                        sc[:, 1, 0:n], k_all[64:128, b, hp, i * 128:(i + 1) * 128],
                        q_fm[64:128, hp, 128 * i:TOK], start=True, stop=True)
                    pt = pp.tile([128, 2, TOK], BF16, tag="p")
                    nc.scalar.activation(pt[:, :, 0:n], sc[:, :, 0:n], AF.Exp)
                    nc.vector.tensor_mul(pt[:, :, 0:128], pt[:, :, 0:128],
                                         msk2[:, b, :, :])
                    return pt

                def emit_av(hp, b, i, pt):
                    n = TOK - 128 * i
                    if b == 0 and i == 0:
                        av[hp] = (psV.tile([128, TOK], F32, tag="av", name=f"ave{l}_{hp}"),
                                  psV.tile([128, TOK], F32, tag="av", name=f"avo{l}_{hp}"))
                    ave, avo = av[hp]
                    st = (b == 0 and i == 0)
                    sp = (b == 1 and i == 3)
                    nc.tensor.matmul(ave[0:HD + 1, 128 * i:TOK],
                                     v_all[:, b, i, 2 * hp, :], pt[:, 0, 0:n],
                                     start=st, stop=sp)
                    nc.tensor.matmul(avo[0:HD + 1, 128 * i:TOK],
                                     v_all[:, b, i, 2 * hp + 1, :], pt[:, 1, 0:n],
                                     start=st, stop=sp)

                def finish_pair(hp):
                    ave, avo = av.pop(hp)
                    den2 = sb.tile([1, 2, TOK], BF16, tag="den")
                    nc.vector.tensor_copy(den2[0:1, 0, :], ave[HD:HD + 1, :])
                    nc.vector.tensor_copy(den2[0:1, 1, :], avo[HD:HD + 1, :])
                    bp = psM.tile([128, TOK], F32, tag="mm")
                    nc.tensor.matmul(bp[0:64, :], onesb[:], den2[0:1, 0, :],
                                     start=True, stop=True)
                    nc.tensor.matmul(bp[64:128, :], onesb[:], den2[0:1, 1, :],
                                     start=True, stop=True)
                    rb = sb.tile([128, TOK], F32, tag="rb")
                    nc.vector.reciprocal_approx_fast(rb[:], bp[:])
                    nc.vector.tensor_tensor(o_fm[0:64, hp, :], ave[0:HD, :],
                                            rb[0:64, :], OP.mult)
                    nc.vector.tensor_tensor(o_fm[64:128, hp, :], avo[0:HD, :],
                                            rb[64:128, :], OP.mult)

                iters = [(hp, b, i) for hp in range(H // 2)
                         for b in range(2) for i in range(4)]
                prev = None
                for j, (hp, b, i) in enumerate(iters):
                    if b == 0 and i == 0 and 3 <= hp <= 6:
                        q_pair(hp + 1)  # deferred Q as PE filler, 1 pair ahead
                    pt = emit_scores(hp, b, i)
                    if prev is not None:
                        emit_av(*prev)
                        if prev[1] == 1 and prev[2] == 3:
                            finish_pair(prev[0])
                    prev = (hp, b, i, pt)
                emit_av(*prev)
                finish_pair(prev[0])

                if l == 0:
                    dbg_dump("k_all0", k_all[:], [128, 2, 8, TOK])
                    dbg_dump("v_all0", v_all[:], [128, 2, 4, H, HD + 1])
                    dbg_dump("o_fm0", o_fm[:], [128, 8, TOK])

                # ---- projection (token-major, t-outer) + residual + LN2(t)
                xn2_fm = actp.tile([128, 8, TOK], BF16, tag="xn2_fm")
                wpt = {}
                for ch in range(2):
                    wt = wch.tile([128, 8, 512], BF16, tag="w", name=f"wpj{l}_{ch}")
                    nc.sync.dma_start(wt[:], wp_h[l, ch])
                    wpt[ch] = wt
                for t in range(4):
                    for ch in range(2):
                        ps = psM.tile([128, 512], F32, tag="mm")
                        for kk in range(8):
                            nc.tensor.matmul(
                                ps[:], o_fm[:, kk, t * 128:(t + 1) * 128],
                                wpt[ch][:, kk, :], start=(kk == 0), stop=(kk == 7))
                        nc.vector.tensor_add(x[:, t, ch * 512:(ch + 1) * 512],
                                             x[:, t, ch * 512:(ch + 1) * 512], ps[:])
                    xn2 = sb.tile([128, 1024], BF16, tag="ln_xn")
                    _ln_tile(nc, sb, stat, t, x, eps, xn2)
                    _ln_transpose(nc, psS, t, xn2, xn2_fm, ident, evac_eng=(t + 1) % 2)
                if l == 0:
                    dbg_dump("xattn0", x[:], [128, 4, D])

                # ---- FFN: ff1 full-token, ff2 in token halves
                h_sb = actp.tile([128, 32, TOK], BF16, tag="h_sb")
                for mc in range(8):
                    wt = wch.tile([128, 8, 512], BF16, tag="w", name=f"w1_{l}_{mc}")
                    nc.sync.dma_start(wt[:], w1_h[l, mc])
                    for mi in range(4):
                        ps = psM.tile([128, TOK], F32, tag="mm")
                        for kk in range(8):
                            nc.tensor.matmul(
                                ps[:], wt[:, kk, mi * 128:(mi + 1) * 128],
                                xn2_fm[:, kk, :], start=(kk == 0), stop=(kk == 7))
                        nc.scalar.activation(h_sb[:, mc * 4 + mi, :], ps[:], AF.Gelu)
                for half in range(2):
                    for nch in range(2):
                        acc = [psV.tile([128, 512], F32, tag="av",
                                        name=f"acc{l}_{half}_{nch}_{a}") for a in range(2)]
                        for kkc in range(8):
                            w2t = wch.tile([128, 4, 512], BF16, tag="w",
                                           name=f"w2_{l}_{half}_{nch}_{kkc}")
                            nc.sync.dma_start(w2t[:], w2_h[l, nch, kkc])
                            for kki in range(4):
                                for mi in range(2):
                                    nc.tensor.matmul(
                                        acc[mi][:],
                                        h_sb[:, kkc * 4 + kki,
                                             half * 256 + mi * 128:half * 256 + (mi + 1) * 128],
                                        w2t[:, kki, :],
                                        start=(kkc == 0 and kki == 0),
                                        stop=(kkc == 7 and kki == 3))
                        for mi in range(2):
                            t = half * 2 + mi
                            nc.vector.tensor_add(x[:, t, nch * 512:(nch + 1) * 512],
                                                 x[:, t, nch * 512:(nch + 1) * 512],
                                                 acc[mi][:])
                if l == 0:
                    dbg_dump("xlayer0", x[:], [128, 4, D])

        # ---- final LN + LM head phase (separate pools; trunk SBUF released)
        with tc.tile_pool(name="stat2", bufs=2) as stat2, \
             tc.tile_pool(name="sb2", bufs=2) as sb2, \
             tc.tile_pool(name="hd", bufs=1) as hd, \
             tc.tile_pool(name="emb", bufs=3) as epool, \
             tc.tile_pool(name="hout", bufs=4) as hout, \
             tc.tile_pool(name="psT2", bufs=2, space="PSUM") as psT2, \
             tc.tile_pool(name="psH", bufs=4, space="PSUM") as psH:
            eps2 = sb2.tile([128, 1], F32, tag="eps", bufs=1)
            nc.vector.memset(eps2[:], EPS)
            xnf_fm = hd.tile([128, 8, TOK], BF16)
            # final LN + world AllGather per token half so head matmuls
            # start on half 0 while half 1 is still in flight
            for t in range(4):
                xnf = sb2.tile([128, 1024], BF16, tag="ln_xn")
                _ln_tile(nc, sb2, stat2, t, x, eps2, xnf)
                _ln_transpose(nc, psT2, t, xnf, xnf_fm, ident, evac_eng=t % 2)
                if t % 2 == 1:
                    hh = t // 2
                    nc.sync.dma_start(
                        agf_in[hh].ap().rearrange("p (kk t1) -> p kk t1", kk=8),
                        xnf_fm[:, :, hh * 256:(hh + 1) * 256])
                    if no_cc:
                        for r_ in range(N_CORES):
                            nc.sync.dma_start(
                                agf_out[hh][r_ * 128:(r_ + 1) * 128, :], agf_in[hh][:])
                    else:
                        nc.gpsimd.collective_compute(
                            "AllGather", OP.bypass, replica_groups=WORLD,
                            ins=[agf_in[hh][:]], outs=[agf_out[hh][:]])
            if dbg:
                dbg_dump("xnf_fm", xnf_fm[:], [128, 8, TOK])
            xn_all = hd.tile([128, 64, TOK], BF16)
            for hh in range(2):
                for r_ in range(8):
                    nc.sync.dma_start(
                        xn_all[:, r_ * 8:(r_ + 1) * 8, hh * 256:(hh + 1) * 256],
                        agf_out[hh][r_ * 128:(r_ + 1) * 128, :].rearrange(
                            "p (kk t1) -> p kk t1", kk=8))
            nchunks = [(i * 512, min(512, VP - i * 512)) for i in range(NVC)]
            for ni, (n0, nsz) in enumerate(nchunks):
                et = epool.tile([128, 8, 512], BF16, tag="emb")
                nc.sync.dma_start(et[:], embT_h[ni])
                # token quarter 0 (tile 0 of every rank) first: available as
                # soon as the first quarter-AG lands
                for t in range(4):
                    for r_ in range(8):
                        mi = r_ * 4 + t
                        ps = psH.tile([128, nsz], F32, tag="h")
                        for kk in range(8):
                            nc.tensor.matmul(
                                ps[:], xn_all[:, r_ * 8 + kk, t * 128:(t + 1) * 128],
                                et[:, kk, 0:nsz],
                                start=(kk == 0), stop=(kk == 7))
                        osb = hout.tile([128, nsz], BF16, tag="o")
                        if (t * 8 + r_) % 2 == 0:
                            nc.vector.tensor_copy(osb[:], ps[:])
                        else:
                            nc.scalar.copy(osb[:], ps[:])
                        nc.sync.dma_start(out_h[mi * 128:(mi + 1) * 128, n0:n0 + nsz], osb[:])

    nc.compile()
    return nc, dbg_outs


def _fm_tile_w(w):
    """[1024, nch*512] -> [nch, 128, 8, 512]; tile[j,p,kk,c] = w[kk*128+p, j*512+c]."""
    din, dout = w.shape
    nch = dout // 512
    r = w.reshape(8, 128, nch, 512)
    return np.ascontiguousarray(r.transpose(2, 1, 0, 3))


def prepare_inputs(idx, tok_emb, pos_emb, qkv_w, qkv_b, proj_w, proj_b,
                   ff1_w, ff1_b, ff2_w, ff2_b, ln1_s, ln1_b, ln2_s, ln2_b,
                   lnf_s, lnf_b, n_layers=L):
    """Host-side sharding/folding. Returns per-core in_maps."""
    bf = ml_dtypes.bfloat16
    for name, v in (("qkv_b", qkv_b), ("proj_b", proj_b), ("ff1_b", ff1_b),
                    ("ff2_b", ff2_b), ("ln1_b", ln1_b), ("ln2_b", ln2_b),
                    ("lnf_b", lnf_b)):
        assert np.allclose(np.asarray(v), 0.0), f"nonzero {name} not supported"

    idx = np.asarray(idx)
    tok_emb = np.asarray(tok_emb, np.float32)
    pos_emb = np.asarray(pos_emb, np.float32)
    scale = 1.0 / np.sqrt(HD)

    # fold LN scales + attention scale into weights (exact)
    wqkv = (np.asarray(qkv_w[:n_layers], np.float32)
            * np.asarray(ln1_s[:n_layers], np.float32)[:, :, None]).copy()
    wqkv[:, :, :D] *= scale
    w1 = (np.asarray(ff1_w[:n_layers], np.float32)
          * np.asarray(ln2_s[:n_layers], np.float32)[:, :, None])
    wp = np.asarray(proj_w[:n_layers], np.float32)
    w2 = np.asarray(ff2_w[:n_layers], np.float32)
    embT_full = (tok_emb * np.asarray(lnf_s, np.float32)[None, :]).T  # [D, V]
    embT_pad = np.zeros((D, N_CORES * VP), np.float32)
    embT_pad[:, :V] = embT_full

    # pre-tiled weight arrays (contiguous 1MB DMA bursts on device)
    wqkv_t = np.stack([_fm_tile_w(wqkv[l]) for l in range(n_layers)]).astype(bf)
    wp_t = np.stack([_fm_tile_w(wp[l]) for l in range(n_layers)]).astype(bf)
    w1_t = np.stack([_fm_tile_w(w1[l]) for l in range(n_layers)]).astype(bf)
    # w2: [4096, 1024] -> [2(nch), 8(kkc), 128(p), 4(kki), 512(c)]
    w2_t = np.stack([
        np.ascontiguousarray(
            w2[l].reshape(8, 4, 128, 2, 512).transpose(3, 0, 2, 1, 4))
        for l in range(n_layers)]).astype(bf)

    ident = np.eye(128, dtype=bf)
    onesb = np.ones((1, 64), bf)

    tri = np.tril(np.ones((128, 128), np.float32)).T  # [kt, q] valid kt<=q
    # core-relative banks: slot0 = local diagonal (triangular for both
    # ranks); slot1 = remote diagonal (all-masked for r=0, visible for r=1)
    msk_r = [np.zeros((2, 128, 128), np.float32) for _ in range(2)]
    msk_r[0][0] = tri
    msk_r[0][1] = 0.0
    msk_r[1][0] = tri
    msk_r[1][1] = 1.0

    in_maps = []
    for c in range(N_CORES):
        b, r = c // 2, c % 2
        pos = positions_for_rank(r)
        x0 = tok_emb[idx[b, pos]] + pos_emb[pos]
        # per-core vocab slice, padded to 13*512 cols for uniform DMA
        esl = np.zeros((D, NVC * 512), np.float32)
        esl[:, :VP] = embT_pad[:, c * VP:(c + 1) * VP]
        embT_tiles = np.ascontiguousarray(
            esl.reshape(8, 128, NVC, 512).transpose(2, 1, 0, 3)).astype(bf)
        in_maps.append({
            "x0": np.ascontiguousarray(x0, np.float32),
            "wqkv": wqkv_t, "wp": wp_t, "w1": w1_t, "w2": w2_t,
            "embT": embT_tiles,
            "msk": msk_r[r].astype(bf),
            "identin": ident,
            "onesb": onesb,
        })
    return in_maps


def assemble_output(results):
    """Per-core [4096, VP] bf16 -> full logits [B, T, V] f32."""
    logits = np.empty((B, T, V), np.float32)
    pos_r = [positions_for_rank(0), positions_for_rank(1)]
    for c in range(N_CORES):
        out_c = np.asarray(results[c]["out"], np.float32)  # [4096, VP]
        v0 = c * VP
        ncols = min(VP, V - v0)
        if ncols <= 0:
            continue
        for r in range(N_CORES):
            bb, rr = r // 2, r % 2
            logits[bb, pos_r[rr], v0:v0 + ncols] = \
                out_c[r * TOK:(r + 1) * TOK, :ncols]
    return logits


_NC_CACHE = {}


def _get_nc(n_layers=L, dbg=False):
    key = (n_layers, dbg)
    if key not in _NC_CACHE:
        _NC_CACHE[key] = build(n_layers=n_layers, dbg=dbg)
    return _NC_CACHE[key]


def kernel(**inputs):
    in_maps = prepare_inputs(**inputs)
    nc, _ = _get_nc()
    res = run_bass_kernel_spmd(nc, in_maps, core_ids=list(range(N_CORES)))
    return assemble_output(res.results)
